# revision 6
# baseline (speedup 1.0000x reference)
"""Distributed TRN2 Bass kernel for one pre-LN transformer decoder layer.

Reference semantics (B=1, T=4096, D=1024, H=16 heads, head=64, FF=4096):
    h  = LN1(x);  qkv = h @ W_qkv + b_qkv;  causal attention;  x += y @ W_o + b_o
    h2 = LN2(x);  x += gelu(h2 @ W1 + b1) @ W2 + b2

Sharding across 8 NeuronCores (SPMD, one static program):
  - sequence-parallel for LN / QKV projection / W_o / MLP: core r owns
    tokens [512r, 512r+512)
  - head-parallel for attention: core r owns heads (2r, 2r+1) - this keeps
    the causal loop bounds/addresses identical on every core (a static SPMD
    graph cannot have per-core trip counts)
  - bridged by three AllToAlls: (Q^T|K^T) bf16, V bf16, y^T f32r

Compute dtypes: projections run fp32r (full PE speed for moving dim >= 256,
no weight cast needed); attention S / exp(S) / PV run bf16. Accumulation is
always fp32 in PSUM. Softmax: scores are bounded (|S| <~ 2 for these LN'd
inputs and 0.02-scaled weights), so exp() without max-subtraction is safe;
the denominator comes for free from a ones-column appended to V. Heads'
attention output is produced directly in feature-major (y^T) layout so the
W_o matmul needs no extra transposes.

`kernel(**inputs)` takes the FULL unsharded inputs and returns the FULL
output; it shards/gathers on the host and runs the compiled NEFF on cores
0-7 via run_bass_kernel_spmd.
"""

import numpy as np
import ml_dtypes

import concourse.bass as bass
import concourse.tile as tile
from concourse import bacc, mybir
from concourse.bass_utils import run_bass_kernel_spmd

F32 = mybir.dt.float32
F32R = mybir.dt.float32r
BF16 = mybir.dt.bfloat16
AF = mybir.ActivationFunctionType
ALU = mybir.AluOpType

T, D, H, HD, DFF = 4096, 1024, 16, 64, 4096
NCORES = 8
LT = T // NCORES          # 512 local tokens per core
P = 128
EPS = 1e-5


def build_nc(repeat: int = 1):
    """Build + compile the SPMD graph (identical on all 8 cores).

    repeat > 1 re-emits the whole layer body N times (for slope-based
    timing); the data path is identical each iteration.
    """
    nc = bacc.Bacc(
        "TRN2",
        target_bir_lowering=False,
        debug=False,
        enable_asserts=True,
        num_devices=NCORES,
    )

    # ---- external I/O (per-core shard shapes) ----
    x_ext = nc.dram_tensor("x", [LT, D], F32, kind="ExternalInput")
    ln1_g = nc.dram_tensor("ln1_g", [D], F32, kind="ExternalInput")
    ln1_b = nc.dram_tensor("ln1_b", [D], F32, kind="ExternalInput")
    ln2_g = nc.dram_tensor("ln2_g", [D], F32, kind="ExternalInput")
    ln2_b = nc.dram_tensor("ln2_b", [D], F32, kind="ExternalInput")
    w_qkv = nc.dram_tensor("W_qkv", [D, 3 * D], F32R, kind="ExternalInput")
    b_qkv = nc.dram_tensor("b_qkv", [3 * D], F32, kind="ExternalInput")
    w_o = nc.dram_tensor("W_o", [D, D], F32R, kind="ExternalInput")
    b_o = nc.dram_tensor("b_o", [D], F32, kind="ExternalInput")
    w_1 = nc.dram_tensor("W1", [D, DFF], F32R, kind="ExternalInput")
    b_1 = nc.dram_tensor("b1", [DFF], F32, kind="ExternalInput")
    w_2 = nc.dram_tensor("W2", [DFF, D], F32R, kind="ExternalInput")
    b_2 = nc.dram_tensor("b2", [D], F32, kind="ExternalInput")
    out_ext = nc.dram_tensor("out", [LT, D], F32, kind="ExternalOutput")

    # ---- internal DRAM (collective bounce buffers) ----
    # A2A block for dest core p: its 2 heads' Q^T/K^T rows, my 512 tokens
    qk_send = nc.dram_tensor("qk_send", [NCORES, 2, P, LT], BF16)
    qk_recv = nc.dram_tensor("qk_recv", [NCORES, 2, P, LT], BF16)
    v_send = nc.dram_tensor("v_send", [NCORES, LT, P], BF16)
    v_recv = nc.dram_tensor("v_recv", [NCORES, LT, P], BF16)
    y_send = nc.dram_tensor("y_send", [NCORES, P, LT], F32R)
    y_recv = nc.dram_tensor("y_recv", [NCORES, P, LT], F32R)
    RG = [list(range(NCORES))]

    # ---- NEFF-embedded constants ----
    # diag masks for 128-row k-chunk m within a 512-wide q tile:
    # keep when (128*m + ki) <= qi
    masks_np = np.stack(
        [
            (128 * m + np.arange(128)[:, None] <= np.arange(512)[None, :])
            for m in range(4)
        ]
    ).astype(ml_dtypes.bfloat16)
    masks_dram = nc.inline_tensor(masks_np, name="diag_masks")
    ident_dram = nc.inline_tensor(np.eye(P, dtype=np.float32), name="ident")

    with tile.TileContext(nc) as tc:
        with tc.tile_pool(name="const", bufs=1) as const:
            eps_t = const.tile([P, 1], F32)
            nc.vector.memset(eps_t, EPS)
            ones_f = const.tile([1, HD], F32)
            nc.vector.memset(ones_f, 1.0)
            ones_col = const.tile([1, HD], F32R)
            nc.scalar.copy(ones_col, ones_f)
            ident = const.tile([P, P], F32)
            nc.sync.dma_start(ident, ident_dram.ap())
            masks = const.tile([P, 4, 512], BF16)
            nc.sync.dma_start(masks, masks_dram.ap().rearrange("m p q -> p m q"))
            g1 = const.tile([P, 8], F32)
            nc.sync.dma_start(g1, ln1_g.ap().rearrange("(s p) -> p s", p=P))
            bb1 = const.tile([P, 8], F32)
            nc.sync.dma_start(bb1, ln1_b.ap().rearrange("(s p) -> p s", p=P))
            g2 = const.tile([P, 8], F32)
            nc.sync.dma_start(g2, ln2_g.ap().rearrange("(s p) -> p s", p=P))
            bb2 = const.tile([P, 8], F32)
            nc.sync.dma_start(bb2, ln2_b.ap().rearrange("(s p) -> p s", p=P))
            bqk = const.tile([P, 16], F32)
            nc.sync.dma_start(bqk, b_qkv.ap()[0 : 2 * D].rearrange("(s p) -> p s", p=P))
            bmlp1 = const.tile([P, 32], F32)
            nc.sync.dma_start(bmlp1, b_1.ap().rearrange("(s p) -> p s", p=P))
            # row-vector biases broadcast across partitions (free-dim biases)
            bv_bc = const.tile([P, D], F32)
            nc.gpsimd.dma_start(
                bv_bc, bass.AP(tensor=b_qkv, offset=2 * D, ap=[[0, P], [1, D]])
            )
            bo_bc = const.tile([P, D], F32)
            nc.gpsimd.dma_start(
                bo_bc, bass.AP(tensor=b_o, offset=0, ap=[[0, P], [1, D]])
            )
            b2_bc = const.tile([P, D], F32)
            nc.gpsimd.dma_start(
                b2_bc, bass.AP(tensor=b_2, offset=0, ap=[[0, P], [1, D]])
            )

            for _rep in range(repeat):
                _layer_body(
                    nc, tc,
                    x_ext, out_ext, w_qkv, w_o, w_1, w_2,
                    qk_send, qk_recv, v_send, v_recv, y_send, y_recv, RG,
                    eps_t, ones_col, ident, masks,
                    g1, bb1, g2, bb2, bqk, bmlp1, bv_bc, bo_bc, b2_bc,
                )

    nc.compile()
    return nc


def _layernorm_to_T(nc, tc, ctx_pools, src_tiles, g_t, b_t, eps_t, ident, dst_T):
    """LN over the feature dim of four [128, 1024] f32 tiles, then transpose
    into feature-major dst_T [128, 8, 512] (f32r), fusing gain/bias into the
    PSUM eviction (they are per-partition there)."""
    tmp, psT = ctx_pools
    for tt in range(4):
        xt = src_tiles[tt]
        stats = tmp.tile([P, 2, 6], F32, tag="lnstats")
        nc.vector.bn_stats(stats[:, 0, :], xt[:, 0:512])
        nc.vector.bn_stats(stats[:, 1, :], xt[:, 512:1024])
        mv = tmp.tile([P, 2], F32, tag="lnmv")
        nc.vector.bn_aggr(mv, stats)
        rsig = tmp.tile([P, 1], F32, tag="lnrsig")
        nc.scalar.activation(rsig, mv[:, 1:2], AF.Sqrt, bias=eps_t)
        nc.vector.reciprocal(rsig, rsig)
        nmu = tmp.tile([P, 1], F32, tag="lnnmu")
        nc.vector.tensor_mul(nmu, mv[:, 0:1], rsig)
        nc.vector.tensor_scalar_mul(nmu, nmu, -1.0)
        ht = tmp.tile([P, D], F32, tag="lnh")
        nc.scalar.activation(ht, xt, AF.Identity, bias=nmu, scale=rsig)
        for i in range(8):
            tp = psT.tile([P, P], F32, tag="lnT")
            nc.tensor.transpose(tp, ht[:, P * i : P * (i + 1)], ident)
            nc.scalar.activation(
                dst_T[:, i, P * tt : P * (tt + 1)],
                tp,
                AF.Identity,
                bias=b_t[:, i : i + 1],
                scale=g_t[:, i : i + 1],
            )


def _layer_body(
    nc, tc,
    x_ext, out_ext, w_qkv, w_o, w_1, w_2,
    qk_send, qk_recv, v_send, v_recv, y_send, y_recv, RG,
    eps_t, ones_col, ident, masks,
    g1, bb1, g2, bb2, bqk, bmlp1, bv_bc, bo_bc, b2_bc,
):
    from contextlib import ExitStack

    with ExitStack() as body:
        resid = body.enter_context(tc.tile_pool(name="resid", bufs=4))
        tmp = body.enter_context(tc.tile_pool(name="tmp", bufs=3))
        hT_pool = body.enter_context(tc.tile_pool(name="hT", bufs=1))

        # ---------- phase 1: load x, LN1 -> h^T ----------
        x_sb = []
        for tt in range(4):
            xt = resid.tile([P, D], F32, tag="x")
            nc.sync.dma_start(xt, x_ext.ap()[P * tt : P * (tt + 1), :])
            x_sb.append(xt)

        hT = hT_pool.tile([P, 8, LT], F32R, tag="hT")
        with tc.tile_pool(name="psT", bufs=2, space="PSUM") as psT:
            _layernorm_to_T(nc, tc, (tmp, psT), x_sb, g1, bb1, eps_t, ident, hT)
            # pre-fold b_o into the residual branch: x + (y@W_o + b_o)
            for tt in range(4):
                nc.vector.tensor_add(x_sb[tt], x_sb[tt], bo_bc)

        # ---------- phase 2: QKV projections ----------
        with tc.tile_pool(name="wqk", bufs=3) as wqk_pool, \
             tc.tile_pool(name="psQK", bufs=2, space="PSUM") as psQK, \
             tc.tile_pool(name="psV", bufs=4, space="PSUM") as psV:
            # Q^T / K^T: f-tile ft covers rows 128ft..(+128) = heads (2ft, 2ft+1)
            for ft in range(16):
                wq = wqk_pool.tile([P, 8, P], F32R, tag="wqk")
                nc.sync.dma_start(
                    wq,
                    w_qkv.ap()[:, P * ft : P * (ft + 1)].rearrange(
                        "(s p) f -> p s f", p=P
                    ),
                )
                ps = psQK.tile([P, LT], F32, tag="qk")
                for k in range(8):
                    nc.tensor.matmul(
                        ps, lhsT=wq[:, k, :], rhs=hT[:, k, :],
                        start=(k == 0), stop=(k == 7),
                    )
                ev = tmp.tile([P, LT], BF16, tag="qkev")
                nc.scalar.activation(ev, ps, AF.Identity, bias=bqk[:, ft : ft + 1])
                if ft < 8:
                    nc.sync.dma_start(qk_send.ap()[ft, 0], ev)
                else:
                    nc.sync.dma_start(qk_send.ap()[ft - 8, 1], ev)
            # V (token-major): out[t, f] accumulated over D
            for n in range(2):
                pvs = [psV.tile([P, LT], F32, tag="vps", name=f"vps{_t}") for _t in range(4)]
                for k in range(8):
                    wv = wqk_pool.tile([P, LT], F32R, tag="wv")
                    nc.sync.dma_start(
                        wv, w_qkv.ap()[P * k : P * (k + 1),
                                       2 * D + LT * n : 2 * D + LT * (n + 1)]
                    )
                    for t in range(4):
                        nc.tensor.matmul(
                            pvs[t], lhsT=hT[:, k, P * t : P * (t + 1)], rhs=wv,
                            start=(k == 0), stop=(k == 7),
                        )
                for t in range(4):
                    vt = tmp.tile([P, LT], BF16, tag="vev")
                    nc.vector.scalar_tensor_tensor(
                        vt, pvs[t], 1.0, bv_bc[:, LT * n : LT * (n + 1)],
                        op0=ALU.mult, op1=ALU.add,
                    )
                    for m in range(4):
                        nc.sync.dma_start(
                            v_send.ap()[4 * n + m, P * t : P * (t + 1), :],
                            vt[:, P * m : P * (m + 1)],
                        )

        # ---------- phase 3: A2As to head-parallel ----------
        nc.gpsimd.collective_compute(
            "AllToAll", ALU.bypass, ins=[qk_send.ap().opt()],
            outs=[qk_recv.ap().opt()], replica_groups=RG,
        )
        nc.gpsimd.collective_compute(
            "AllToAll", ALU.bypass, ins=[v_send.ap().opt()],
            outs=[v_recv.ap().opt()], replica_groups=RG,
        )

        # ---------- phase 4: causal attention for my 2 heads ----------
        with tc.tile_pool(name="att", bufs=3) as att, \
             tc.tile_pool(name="psS", bufs=2, space="PSUM") as psS, \
             tc.tile_pool(name="psO", bufs=2, space="PSUM") as psO, \
             tc.tile_pool(name="psB", bufs=1, space="PSUM") as psB:
            for qt in range(8):
                q_sb = att.tile([P, LT], BF16, tag="q")
                nc.sync.dma_start(q_sb, qk_recv.ap()[qt, 0])
                o_ps = [psO.tile([HD + 1, LT], F32, tag=f"o{h}", name=f"o{h}") for h in range(2)]
                nk = 4 * qt + 4
                for j in range(nk):
                    k_sb = att.tile([P, P], BF16, tag="k")
                    nc.sync.dma_start(
                        k_sb,
                        qk_recv.ap()[j // 4, 1, :, P * (j % 4) : P * (j % 4 + 1)],
                    )
                    v_sb = att.tile([P, 2, HD + 1], BF16, tag="v")
                    nc.sync.dma_start(
                        v_sb[:, :, 0:HD],
                        v_recv.ap()[j // 4, P * (j % 4) : P * (j % 4 + 1), :]
                        .rearrange("p (h d) -> p h d", h=2),
                    )
                    nc.vector.memset(v_sb[:, :, HD : HD + 1], 1.0)
                    for h in range(2):
                        s_ps = psS.tile([P, LT], F32, tag="s")
                        nc.tensor.matmul(
                            s_ps,
                            lhsT=k_sb[HD * h : HD * (h + 1), :],
                            rhs=q_sb[HD * h : HD * (h + 1), :],
                            start=True, stop=True,
                        )
                        es = att.tile([P, LT], BF16, tag="es")
                        nc.scalar.activation(es, s_ps, AF.Exp, scale=0.125)
                        if j >= nk - 4:
                            em = att.tile([P, LT], BF16, tag="em")
                            nc.vector.tensor_mul(em, es, masks[:, j - (nk - 4), :])
                            es = em
                        nc.tensor.matmul(
                            o_ps[h], lhsT=v_sb[:, h, :], rhs=es,
                            start=(j == 0), stop=(j == nk - 1),
                        )
                # normalize: y^T = O / denom (denom = ones-column row of o_ps)
                y_sb = att.tile([P, LT], F32R, tag="y")
                for h in range(2):
                    rec = att.tile([1, LT], F32R, tag="rec")
                    with nc.allow_low_precision(reason="f32r softmax denom"):
                        nc.vector.reciprocal(rec, o_ps[h][HD : HD + 1, :])
                    bc_ps = psB.tile([HD, LT], F32, tag="bc")
                    nc.tensor.matmul(
                        bc_ps, lhsT=ones_col, rhs=rec, start=True, stop=True
                    )
                    bc_sb = att.tile([HD, LT], F32, tag="bcs")
                    nc.vector.tensor_copy(bc_sb, bc_ps)
                    nc.vector.tensor_mul(
                        y_sb[HD * h : HD * (h + 1), :], o_ps[h][0:HD, :], bc_sb
                    )
                nc.sync.dma_start(y_send.ap()[qt], y_sb)

        # ---------- phase 5: y back to sequence-parallel ----------
        nc.gpsimd.collective_compute(
            "AllToAll", ALU.bypass, ins=[y_send.ap().opt()],
            outs=[y_recv.ap().opt()], replica_groups=RG,
        )

        # ---------- phase 6: W_o + residual ----------
        x_att = []
        for tt in range(4):
            x_att.append(resid.tile([P, D], F32, tag="xatt", name=f"xatt{tt}"))
        with tc.tile_pool(name="yT", bufs=1) as yT_pool, \
             tc.tile_pool(name="wo", bufs=3) as wo_pool, \
             tc.tile_pool(name="psAt", bufs=4, space="PSUM") as psAt:
            yT = yT_pool.tile([P, 8, LT], F32R, tag="yT")
            nc.sync.dma_start(yT, y_recv.ap().rearrange("i p t -> p i t"))
            for n in range(2):
                pats = [psAt.tile([P, LT], F32, tag="att", name=f"att{_t}") for _t in range(4)]
                for k in range(8):
                    wo_t = wo_pool.tile([P, LT], F32R, tag="wo")
                    nc.sync.dma_start(
                        wo_t, w_o.ap()[P * k : P * (k + 1), LT * n : LT * (n + 1)]
                    )
                    for t in range(4):
                        nc.tensor.matmul(
                            pats[t], lhsT=yT[:, k, P * t : P * (t + 1)], rhs=wo_t,
                            start=(k == 0), stop=(k == 7),
                        )
                for t in range(4):
                    nc.vector.tensor_add(
                        x_att[t][:, LT * n : LT * (n + 1)], pats[t],
                        x_sb[t][:, LT * n : LT * (n + 1)],
                    )

        # ---------- phase 7: LN2 -> h2^T ----------
        h2T_pool = body.enter_context(tc.tile_pool(name="h2T", bufs=1))
        h2T = h2T_pool.tile([P, 8, LT], F32R, tag="h2T")
        with tc.tile_pool(name="psT2", bufs=2, space="PSUM") as psT2:
            _layernorm_to_T(nc, tc, (tmp, psT2), x_att, g2, bb2, eps_t, ident, h2T)
            for tt in range(4):
                nc.vector.tensor_add(x_att[tt], x_att[tt], b2_bc)

        # ---------- phase 8: MLP1 (gelu(h2 @ W1 + b1))^T ----------
        with tc.tile_pool(name="gT", bufs=1) as gT_pool, \
             tc.tile_pool(name="w1p", bufs=3) as w1_pool, \
             tc.tile_pool(name="psM1", bufs=2, space="PSUM") as psM1:
            gT = gT_pool.tile([P, 32, LT], F32R, tag="gT")
            for m in range(32):
                w1_t = w1_pool.tile([P, 8, P], F32R, tag="w1")
                nc.sync.dma_start(
                    w1_t,
                    w_1.ap()[:, P * m : P * (m + 1)].rearrange("(s p) f -> p s f", p=P),
                )
                ps = psM1.tile([P, LT], F32, tag="m1")
                for k in range(8):
                    nc.tensor.matmul(
                        ps, lhsT=w1_t[:, k, :], rhs=h2T[:, k, :],
                        start=(k == 0), stop=(k == 7),
                    )
                nc.scalar.activation(
                    gT[:, m, :], ps, AF.Gelu, bias=bmlp1[:, m : m + 1]
                )

            # ---------- phase 9: MLP2 + residual -> out ----------
            with tc.tile_pool(name="w2p", bufs=3) as w2_pool, \
                 tc.tile_pool(name="psM2", bufs=4, space="PSUM") as psM2:
                for n in range(2):
                    pms = [psM2.tile([P, LT], F32, tag="m2", name=f"m2{_t}") for _t in range(4)]
                    for k in range(32):
                        w2_t = w2_pool.tile([P, LT], F32R, tag="w2")
                        nc.sync.dma_start(
                            w2_t, w_2.ap()[P * k : P * (k + 1), LT * n : LT * (n + 1)]
                        )
                        for t in range(4):
                            nc.tensor.matmul(
                                pms[t], lhsT=gT[:, k, P * t : P * (t + 1)], rhs=w2_t,
                                start=(k == 0), stop=(k == 31),
                            )
                    for t in range(4):
                        ot = tmp.tile([P, LT], F32, tag="outev")
                        nc.vector.tensor_add(
                            ot, pms[t], x_att[t][:, LT * n : LT * (n + 1)]
                        )
                        nc.sync.dma_start(
                            out_ext.ap()[P * t : P * (t + 1), LT * n : LT * (n + 1)],
                            ot,
                        )


_NC_CACHE = {}


def _get_nc(repeat: int = 1):
    if repeat not in _NC_CACHE:
        _NC_CACHE[repeat] = build_nc(repeat)
    return _NC_CACHE[repeat]


def make_in_maps(inputs: dict) -> list:
    arr = {k: np.ascontiguousarray(np.asarray(v)) for k, v in inputs.items()}
    x = arr["x"].astype(np.float32, copy=False).reshape(T, D)
    weights = {
        k: arr[k].astype(np.float32, copy=False)
        for k in (
            "ln1_g", "ln1_b", "ln2_g", "ln2_b", "W_qkv", "b_qkv",
            "W_o", "b_o", "W1", "b1", "W2", "b2",
        )
    }
    in_maps = []
    for r in range(NCORES):
        m = {"x": np.ascontiguousarray(x[LT * r : LT * (r + 1)])}
        m.update(weights)
        in_maps.append(m)
    return in_maps


def kernel(**inputs) -> np.ndarray:
    am = np.asarray(inputs["attention_mask"])
    assert np.all(am != 0), "kernel assumes an all-ones attention mask"
    nc = _get_nc(1)
    in_maps = make_in_maps(inputs)
    res = run_bass_kernel_spmd(nc, in_maps, core_ids=list(range(NCORES)))
    out = np.empty((T, D), np.float32)
    for r in range(NCORES):
        out[LT * r : LT * (r + 1)] = res.results[r]["out"]
    return out.reshape(1, T, D)


# revision 22
# speedup vs baseline: 399.2479x; 399.2479x over previous
"""Distributed TRN2 Bass kernel for one pre-LN transformer decoder layer.

Reference semantics (B=1, T=4096, D=1024, H=16 heads, head=64, FF=4096):
    h  = LN1(x);  qkv = h @ W_qkv + b_qkv;  causal attention;  x += y @ W_o + b_o
    h2 = LN2(x);  x += gelu(h2 @ W1 + b1) @ W2 + b2

Sharding across 8 NeuronCores (SPMD, one static program):
  - sequence-parallel for LN / QKV projection / W_o / MLP: core r owns
    tokens [512r, 512r+512)
  - head-parallel for attention: core r owns heads (2r, 2r+1) - this keeps
    the causal loop bounds/addresses identical on every core (a static SPMD
    graph cannot have per-core trip counts)
  - bridged by three AllToAlls: (Q^T|K^T) bf16, V bf16, y^T f32r

Compute dtypes: projections run fp32r (full PE speed for moving dim >= 256,
no weight cast needed); attention S / exp(S) / PV run bf16. Accumulation is
always fp32 in PSUM. Softmax: scores are bounded (|S| <~ 2 for these LN'd
inputs and 0.02-scaled weights), so exp() without max-subtraction is safe;
the denominator comes for free from a ones-column appended to V. Heads'
attention output is produced directly in feature-major (y^T) layout so the
W_o matmul needs no extra transposes.

Attention inner loop processes one source core's 512-token block per DMA
(K^T [128,512] contiguous, V via one strided DMA), and the two heads' S
matmuls target distinct PE row-groups (lhsT base partitions 0 / 64) so the
hardware runs them concurrently; exp() is one ACT op over both heads' S.

`kernel(**inputs)` takes the FULL unsharded inputs and returns the FULL
output; it shards/gathers on the host and runs the compiled NEFF on cores
0-7 via run_bass_kernel_spmd.
"""

import numpy as np
import ml_dtypes

import concourse.bass as bass
import concourse.tile as tile
from concourse import bacc, mybir
from concourse.bass_utils import run_bass_kernel_spmd

F32 = mybir.dt.float32
F32R = mybir.dt.float32r
BF16 = mybir.dt.bfloat16
FP8 = mybir.dt.float8e4
DR = mybir.MatmulPerfMode.DoubleRow
AF = mybir.ActivationFunctionType
ALU = mybir.AluOpType

T, D, H, HD, DFF = 4096, 1024, 16, 64, 4096
NCORES = 8
LT = T // NCORES          # 512 local tokens per core
P = 128
EPS = 1e-5


def build_nc(repeat: int = 1):
    """Build + compile the SPMD graph (identical on all 8 cores).

    repeat > 1 re-emits the whole layer body N times (for slope-based
    timing); the data path is identical each iteration.
    """
    nc = bacc.Bacc(
        "TRN2",
        target_bir_lowering=False,
        debug=False,
        enable_asserts=True,
        num_devices=NCORES,
    )

    # ---- external I/O (per-core shard shapes) ----
    x_ext = nc.dram_tensor("x", [LT, D], F32, kind="ExternalInput")
    ln1_g = nc.dram_tensor("ln1_g", [D], F32, kind="ExternalInput")
    ln1_b = nc.dram_tensor("ln1_b", [D], F32, kind="ExternalInput")
    ln2_g = nc.dram_tensor("ln2_g", [D], F32, kind="ExternalInput")
    ln2_b = nc.dram_tensor("ln2_b", [D], F32, kind="ExternalInput")
    w_qkv = nc.dram_tensor("W_qkv", [D, 3 * D], F32R, kind="ExternalInput")
    b_qkv = nc.dram_tensor("b_qkv", [3 * D], F32, kind="ExternalInput")
    w_o = nc.dram_tensor("W_o", [D, D], F32R, kind="ExternalInput")
    b_o = nc.dram_tensor("b_o", [D], F32, kind="ExternalInput")
    w_1 = nc.dram_tensor("W1", [D, DFF], F32R, kind="ExternalInput")
    b_1 = nc.dram_tensor("b1", [DFF], F32, kind="ExternalInput")
    w_2 = nc.dram_tensor("W2", [DFF, D], F32R, kind="ExternalInput")
    b_2 = nc.dram_tensor("b2", [D], F32, kind="ExternalInput")
    out_ext = nc.dram_tensor("out", [LT, D], F32, kind="ExternalOutput")

    # ---- internal DRAM (collective bounce buffers) ----
    # A2A block for dest core p: its 2 heads' Q^T/K^T rows, my 512 tokens
    qk_send = nc.dram_tensor("qk_send", [NCORES, 2, P, LT], BF16)
    qk_recv = nc.dram_tensor("qk_recv", [NCORES, 2, P, LT], BF16)
    v_send = nc.dram_tensor("v_send", [NCORES, LT, P], BF16)
    v_recv = nc.dram_tensor("v_recv", [NCORES, LT, P], BF16)
    y_send = nc.dram_tensor("y_send", [NCORES, P, LT], BF16)
    y_recv = nc.dram_tensor("y_recv", [NCORES, P, LT], BF16)
    RG = [list(range(NCORES))]

    # ---- NEFF-embedded constants ----
    # diag masks for 128-row k-chunk m within a 512-wide q tile:
    # keep when (128*m + ki) <= qi
    masks_np = np.stack(
        [
            (128 * m + np.arange(128)[:, None] <= np.arange(512)[None, :])
            for m in range(4)
        ]
    ).astype(ml_dtypes.bfloat16)
    masks_dram = nc.inline_tensor(masks_np, name="diag_masks")
    ident_dram = nc.inline_tensor(np.eye(P, dtype=np.float32), name="ident")

    with tile.TileContext(nc) as tc:
        with tc.tile_pool(name="const", bufs=1) as const:
            eps_t = const.tile([P, 1], F32)
            nc.vector.memset(eps_t, EPS)
            ones_f = const.tile([1, HD], F32)
            nc.vector.memset(ones_f, 1.0)
            ones_col = const.tile([1, HD], F32R)
            nc.scalar.copy(ones_col, ones_f)
            ident = const.tile([P, P], F32)
            nc.sync.dma_start(ident, ident_dram.ap())
            masks = const.tile([P, 4, 512], BF16)
            nc.sync.dma_start(masks, masks_dram.ap().rearrange("m p q -> p m q"))
            g1 = const.tile([P, 8], F32)
            nc.sync.dma_start(g1, ln1_g.ap().rearrange("(s p) -> p s", p=P))
            bb1 = const.tile([P, 8], F32)
            nc.sync.dma_start(bb1, ln1_b.ap().rearrange("(s p) -> p s", p=P))
            g2 = const.tile([P, 8], F32)
            nc.sync.dma_start(g2, ln2_g.ap().rearrange("(s p) -> p s", p=P))
            bb2 = const.tile([P, 8], F32)
            nc.sync.dma_start(bb2, ln2_b.ap().rearrange("(s p) -> p s", p=P))
            bqk = const.tile([P, 16], F32)
            nc.sync.dma_start(bqk, b_qkv.ap()[0 : 2 * D].rearrange("(s p) -> p s", p=P))
            bmlp1 = const.tile([P, 32], F32)
            nc.sync.dma_start(bmlp1, b_1.ap().rearrange("(s p) -> p s", p=P))
            # row-vector biases broadcast across partitions (free-dim biases)
            bv_bc = const.tile([P, D], F32)
            nc.gpsimd.dma_start(
                bv_bc, bass.AP(tensor=b_qkv, offset=2 * D, ap=[[0, P], [1, D]])
            )
            bo_bc = const.tile([P, D], F32)
            nc.gpsimd.dma_start(
                bo_bc, bass.AP(tensor=b_o, offset=0, ap=[[0, P], [1, D]])
            )
            b2_bc = const.tile([P, D], F32)
            nc.gpsimd.dma_start(
                b2_bc, bass.AP(tensor=b_2, offset=0, ap=[[0, P], [1, D]])
            )

            for _rep in range(repeat):
                _layer_body(
                    nc, tc,
                    x_ext, out_ext, w_qkv, w_o, w_1, w_2,
                    qk_send, qk_recv, v_send, v_recv, y_send, y_recv, RG,
                    eps_t, ones_col, ident, masks,
                    g1, bb1, g2, bb2, bqk, bmlp1, bv_bc, bo_bc, b2_bc,
                )

    nc.compile()
    return nc


def _layernorm_to_T(nc, tc, ctx_pools, src_tiles, g_t, b_t, eps_t, ident, dst_T):
    """LN over the feature dim of four [128, 1024] f32 tiles, then transpose
    into feature-major dst_T [128, 8, 512], fusing gain/bias into the PSUM
    eviction (per-partition there). Stats+apply run for all four tiles
    first; the transposes then go k-chunk-outer so dst_T[:, 0, :] is ready
    early and downstream matmuls can begin while the rest transpose."""
    tmp, psT, hpool = ctx_pools
    hts = []
    for tt in range(4):
        xt = src_tiles[tt]
        stats = tmp.tile([P, 2, 6], F32, tag="lnstats")
        nc.vector.bn_stats(stats[:, 0, :], xt[:, 0:512])
        nc.vector.bn_stats(stats[:, 1, :], xt[:, 512:1024])
        mv = tmp.tile([P, 2], F32, tag="lnmv")
        nc.vector.bn_aggr(mv, stats)
        rsig = tmp.tile([P, 1], F32, tag="lnrsig")
        nc.scalar.activation(rsig, mv[:, 1:2], AF.Sqrt, bias=eps_t)
        nc.vector.reciprocal(rsig, rsig)
        nmu = tmp.tile([P, 1], F32, tag="lnnmu")
        nc.vector.tensor_mul(nmu, mv[:, 0:1], rsig)
        nc.vector.tensor_scalar_mul(nmu, nmu, -1.0)
        ht = hpool.tile([P, D], F32, tag="lnh", name=f"lnh{tt}")
        nc.scalar.activation(ht, xt, AF.Identity, bias=nmu, scale=rsig)
        hts.append(ht)
    for i in range(8):
        for tt in range(4):
            tp = psT.tile([P, P], F32, tag="lnT")
            nc.tensor.transpose(tp, hts[tt][:, P * i : P * (i + 1)], ident)
            nc.scalar.activation(
                dst_T[:, i, P * tt : P * (tt + 1)],
                tp,
                AF.Identity,
                bias=b_t[:, i : i + 1],
                scale=g_t[:, i : i + 1],
            )


def _layer_body(
    nc, tc,
    x_ext, out_ext, w_qkv, w_o, w_1, w_2,
    qk_send, qk_recv, v_send, v_recv, y_send, y_recv, RG,
    eps_t, ones_col, ident, masks,
    g1, bb1, g2, bb2, bqk, bmlp1, bv_bc, bo_bc, b2_bc,
):
    from contextlib import ExitStack

    with ExitStack() as body:
        resid = body.enter_context(tc.tile_pool(name="resid", bufs=4))
        tmp = body.enter_context(tc.tile_pool(name="tmp", bufs=4))
        hT_pool = body.enter_context(tc.tile_pool(name="hT", bufs=1))

        # ---------- phase 1: load x, LN1 -> h^T ----------
        x_sb = []
        for tt in range(4):
            xt = resid.tile([P, D], F32, tag="x", name=f"x{tt}")
            nc.sync.dma_start(xt, x_ext.ap()[P * tt : P * (tt + 1), :])
            x_sb.append(xt)

        hT = hT_pool.tile([P, 8, LT], BF16, tag="hT")
        with tc.tile_pool(name="psT", bufs=2, space="PSUM") as psT, \
             tc.tile_pool(name="lnh1", bufs=4) as hp1:
            _layernorm_to_T(nc, tc, (tmp, psT, hp1), x_sb, g1, bb1, eps_t, ident, hT)
            # pre-fold b_o into the residual branch: x + (y@W_o + b_o)
            for tt in range(4):
                nc.vector.tensor_add(x_sb[tt], x_sb[tt], bo_bc)

        # ---------- phase 2: QKV projections ----------
        with tc.tile_pool(name="wqk", bufs=4) as wqk_pool:
            # Q^T / K^T: block fb covers rows 256fb..(+256) = f-tiles (2fb, 2fb+1)
            psQK_ctx = tc.tile_pool(name="psQK", bufs=2, space="PSUM")
            psQK = psQK_ctx.__enter__()
            for fb in range(8):
                wq = wqk_pool.tile([P, 8, 256], BF16, tag="wqk")
                nc.gpsimd.dma_start(
                    wq,
                    w_qkv.ap()[:, 256 * fb : 256 * (fb + 1)].rearrange(
                        "(s p) f -> p s f", p=P
                    ),
                )
                for half in range(2):
                    ft = 2 * fb + half
                    ps = psQK.tile([P, LT], F32, tag="qk")
                    for k in range(8):
                        nc.tensor.matmul(
                            ps,
                            lhsT=wq[:, k, P * half : P * (half + 1)],
                            rhs=hT[:, k, :],
                            start=(k == 0), stop=(k == 7),
                        )
                    ev = tmp.tile([P, LT], BF16, tag="qkev")
                    nc.vector.tensor_scalar_add(ev, ps, bqk[:, ft : ft + 1])
                    if ft < 8:
                        nc.sync.dma_start(qk_send.ap()[ft, 0], ev)
                    else:
                        nc.sync.dma_start(qk_send.ap()[ft - 8, 1], ev)
            # V (token-major): out[t, f] accumulated over D
            psQK_ctx.__exit__(None, None, None)
            psV_ctx = tc.tile_pool(name="psV", bufs=1, space="PSUM")
            psV = psV_ctx.__enter__()
            pvs = [
                psV.tile([P, 2, LT], F32, tag=f"vps{_t}", name=f"vps{_t}")
                for _t in range(4)
            ]
            for k in range(8):
                wv = wqk_pool.tile([P, D], BF16, tag="wv")
                nc.gpsimd.dma_start(
                    wv, w_qkv.ap()[P * k : P * (k + 1), 2 * D : 3 * D]
                )
                for t in range(4):
                    for n in range(2):
                        nc.tensor.matmul(
                            pvs[t][:, n, :],
                            lhsT=hT[:, k, P * t : P * (t + 1)],
                            rhs=wv[:, LT * n : LT * (n + 1)],
                            start=(k == 0), stop=(k == 7),
                        )
            for t in range(4):
                vt = tmp.tile([P, D], BF16, tag="vev")
                nc.vector.scalar_tensor_tensor(
                    vt, pvs[t].rearrange("p n f -> p (n f)"), 1.0, bv_bc,
                    op0=ALU.mult, op1=ALU.add,
                )
                # one strided DMA scatters the 8 dest-core column blocks
                nc.sync.dma_start(
                    v_send.ap()[:, P * t : P * (t + 1), :].rearrange(
                        "m p f -> p m f"
                    ),
                    vt.rearrange("p (m f) -> p m f", f=P),
                )
            psV_ctx.__exit__(None, None, None)

        # ---------- phase 3: A2As to head-parallel ----------
        nc.gpsimd.collective_compute(
            "AllToAll", ALU.bypass, ins=[qk_send.ap().opt()],
            outs=[qk_recv.ap().opt()], replica_groups=RG,
        )
        nc.gpsimd.collective_compute(
            "AllToAll", ALU.bypass, ins=[v_send.ap().opt()],
            outs=[v_recv.ap().opt()], replica_groups=RG,
        )

        # ---------- phase 4: causal attention for my 2 heads ----------
        with tc.tile_pool(name="att", bufs=4) as att, \
             tc.tile_pool(name="esp", bufs=6) as esp, \
             tc.tile_pool(name="psS", bufs=3, space="PSUM") as psS, \
             tc.tile_pool(name="psO", bufs=1, space="PSUM") as psO:
            for qt in range(8):
                q_sb = att.tile([P, LT], BF16, tag="q")
                nc.sync.dma_start(q_sb, qk_recv.ap()[qt, 0])
                o_ps = psO.tile([P, 2, LT], F32, tag="o", name=f"o{qt}")
                for b in range(qt + 1):
                    k_sb = att.tile([P, LT], BF16, tag="k")
                    nc.sync.dma_start(k_sb, qk_recv.ap()[b, 1])
                    v_sb = att.tile([P, 8, 2 * HD], BF16, tag="v")
                    for h in range(2):
                        nc.sync.dma_start(
                            v_sb[:, 4 * h : 4 * (h + 1), 0:HD],
                            v_recv.ap()[b][:, HD * h : HD * (h + 1)].rearrange(
                                "(a p) d -> p a d", p=P
                            ),
                        )
                    nc.gpsimd.memset(v_sb[:, :, HD : 2 * HD], 1.0)
                    for jj in range(4):
                        j_first = b == 0 and jj == 0
                        j_last = b == qt and jj == 3
                        # on the diagonal block only columns >= 128jj matter
                        q0 = P * jj if b == qt else 0
                        w = LT - q0
                        s_ps = psS.tile([P, 2, LT], F32, tag="s")
                        # the two heads' lhsT live at base partitions 0 / 64
                        # -> distinct PE row-groups, hardware-concurrent
                        for h in range(2):
                            nc.tensor.matmul(
                                s_ps[:, h, q0:LT],
                                lhsT=k_sb[HD * h : HD * (h + 1),
                                          P * jj : P * (jj + 1)],
                                rhs=q_sb[HD * h : HD * (h + 1), q0:LT],
                                start=True, stop=True,
                            )
                        es = esp.tile([P, 2, LT], BF16, tag="es")
                        nc.scalar.activation(
                            es[:, :, q0:LT], s_ps[:, :, q0:LT], AF.Exp, scale=0.125
                        )
                        if b == qt:
                            em = esp.tile([P, 2, LT], BF16, tag="em")
                            # masks[0][:, 0:w] is exactly the triangle pattern
                            # of diag chunk jj restricted to its live columns
                            nc.vector.tensor_mul(
                                em[:, :, q0:LT], es[:, :, q0:LT],
                                masks[:, 0:1, 0:w].broadcast_to([P, 2, w]),
                            )
                            es = em
                        for h in range(2):
                            nc.tensor.matmul(
                                o_ps[:, h, q0:LT], lhsT=v_sb[:, 4 * h + jj, :],
                                rhs=es[:, h, q0:LT],
                                start=j_first, stop=j_last,
                            )
                # normalize: y^T = O / denom (denom = ones-column row of o_ps)
                y_sb = att.tile([P, LT], BF16, tag="y")
                for h in range(2):
                    rec = att.tile([HD, LT], F32, tag="rec")
                    nc.vector.reciprocal(rec, o_ps[HD : 2 * HD, h, :])
                    nc.vector.tensor_mul(
                        y_sb[HD * h : HD * (h + 1), :], o_ps[0:HD, h, :], rec
                    )
                nc.sync.dma_start(y_send.ap()[qt], y_sb)

        # ---------- phase 5: y back to sequence-parallel ----------
        nc.gpsimd.collective_compute(
            "AllToAll", ALU.bypass, ins=[y_send.ap().opt()],
            outs=[y_recv.ap().opt()], replica_groups=RG,
        )

        # ---------- phase 6: W_o + residual ----------
        x_att = []
        for tt in range(4):
            x_att.append(resid.tile([P, D], F32, tag="xatt", name=f"xatt{tt}"))
        with tc.tile_pool(name="yT", bufs=1) as yT_pool, \
             tc.tile_pool(name="wo", bufs=4) as wo_pool, \
             tc.tile_pool(name="psAt", bufs=4, space="PSUM") as psAt:
            yT = yT_pool.tile([P, 8, LT], BF16, tag="yT")
            nc.sync.dma_start(yT, y_recv.ap().rearrange("i p t -> p i t"))
            for n in range(2):
                pats = [
                    psAt.tile([P, LT], F32, tag="att", name=f"att{_t}")
                    for _t in range(4)
                ]
                for k in range(4):
                    wo_t = wo_pool.tile([P, 2, LT], BF16, tag="wo")
                    nc.gpsimd.dma_start(
                        wo_t,
                        w_o.ap()[256 * k : 256 * (k + 1),
                                 LT * n : LT * (n + 1)].rearrange(
                            "(s p) f -> p s f", p=P
                        ),
                    )
                    for s in range(2):
                        for t in range(4):
                            nc.tensor.matmul(
                                pats[t],
                                lhsT=yT[:, 2 * k + s, P * t : P * (t + 1)],
                                rhs=wo_t[:, s, :],
                                start=(k == 0 and s == 0),
                                stop=(k == 3 and s == 1),
                            )
                for t in range(4):
                    nc.vector.tensor_add(
                        x_att[t][:, LT * n : LT * (n + 1)], pats[t],
                        x_sb[t][:, LT * n : LT * (n + 1)],
                    )

        # ---------- phase 7: LN2 -> h2^T ----------
        h2T_pool = body.enter_context(tc.tile_pool(name="h2T", bufs=1))
        h2T = h2T_pool.tile([P, 8, LT], BF16, tag="h2T")
        with tc.tile_pool(name="psT2", bufs=2, space="PSUM") as psT2, \
             tc.tile_pool(name="lnh2", bufs=4) as hp2:
            _layernorm_to_T(nc, tc, (tmp, psT2, hp2), x_att, g2, bb2, eps_t, ident, h2T)
            for tt in range(4):
                nc.vector.tensor_add(x_att[tt], x_att[tt], b2_bc)

        # ---------- phase 8: MLP1 (gelu(h2 @ W1 + b1))^T ----------
        gT_pool = body.enter_context(tc.tile_pool(name="gT", bufs=1))
        gT = gT_pool.tile([P, 32, LT], BF16, tag="gT")
        with tc.tile_pool(name="w1p", bufs=4) as w1_pool, \
             tc.tile_pool(name="psM1", bufs=2, space="PSUM") as psM1:
            for mb in range(16):
                w1_t = w1_pool.tile([P, 8, 256], BF16, tag="w1")
                nc.gpsimd.dma_start(
                    w1_t,
                    w_1.ap()[:, 256 * mb : 256 * (mb + 1)].rearrange(
                        "(s p) f -> p s f", p=P
                    ),
                )
                for half in range(2):
                    m = 2 * mb + half
                    ps = psM1.tile([P, LT], F32, tag="m1")
                    for k in range(8):
                        nc.tensor.matmul(
                            ps,
                            lhsT=w1_t[:, k, P * half : P * (half + 1)],
                            rhs=h2T[:, k, :],
                            start=(k == 0), stop=(k == 7),
                        )
                    nc.scalar.activation(
                        gT[:, m, :], ps, AF.Gelu, bias=bmlp1[:, m : m + 1]
                    )

        # ---------- phase 9: MLP2 + residual -> out ----------
        with tc.tile_pool(name="w2p", bufs=4) as w2_pool, \
             tc.tile_pool(name="psM2", bufs=1, space="PSUM") as psM2:
            pms = [
                psM2.tile([P, LT], F32, tag=f"m2_{_n}_{_t}", name=f"m2_{_n}_{_t}")
                for _n in range(2) for _t in range(4)
            ]
            for k in range(16):
                w2_t = w2_pool.tile([P, 2, D], BF16, tag="w2")
                nc.gpsimd.dma_start(
                    w2_t,
                    w_2.ap()[256 * k : 256 * (k + 1), :].rearrange(
                        "(s p) f -> p s f", p=P
                    ),
                )
                for s in range(2):
                    for n in range(2):
                        for t in range(4):
                            nc.tensor.matmul(
                                pms[4 * n + t],
                                lhsT=gT[:, 2 * k + s, P * t : P * (t + 1)],
                                rhs=w2_t[:, s, LT * n : LT * (n + 1)],
                                start=(k == 0 and s == 0),
                                stop=(k == 15 and s == 1),
                            )
            for n in range(2):
                for t in range(4):
                    ot = tmp.tile([P, LT], F32, tag="outev")
                    nc.vector.tensor_add(
                        ot, pms[4 * n + t], x_att[t][:, LT * n : LT * (n + 1)]
                    )
                    nc.sync.dma_start(
                        out_ext.ap()[P * t : P * (t + 1), LT * n : LT * (n + 1)],
                        ot,
                    )


_NC_CACHE = {}


def _get_nc(repeat: int = 1):
    if repeat not in _NC_CACHE:
        _NC_CACHE[repeat] = build_nc(repeat)
    return _NC_CACHE[repeat]


def make_in_maps(inputs: dict) -> list:
    arr = {k: np.ascontiguousarray(np.asarray(v)) for k, v in inputs.items()}
    x = arr["x"].astype(np.float32, copy=False).reshape(T, D)
    weights = {
        k: arr[k].astype(np.float32, copy=False)
        for k in (
            "ln1_g", "ln1_b", "ln2_g", "ln2_b", "W_qkv", "b_qkv",
            "W_o", "b_o", "W1", "b1", "W2", "b2",
        )
    }
    in_maps = []
    for r in range(NCORES):
        m = {"x": np.ascontiguousarray(x[LT * r : LT * (r + 1)])}
        m.update(weights)
        in_maps.append(m)
    return in_maps


def kernel(**inputs) -> np.ndarray:
    am = np.asarray(inputs["attention_mask"])
    assert np.all(am != 0), "kernel assumes an all-ones attention mask"
    nc = _get_nc(1)
    in_maps = make_in_maps(inputs)
    last_err = None
    for attempt in range(3):
        try:
            res = run_bass_kernel_spmd(nc, in_maps, core_ids=list(range(NCORES)))
            break
        except Exception as e:  # transient device wedges recover on retry
            last_err = e
            import time as _time

            _time.sleep(10)
    else:
        raise last_err
    out = np.empty((T, D), np.float32)
    for r in range(NCORES):
        out[LT * r : LT * (r + 1)] = res.results[r]["out"]
    return out.reshape(1, T, D)
